# revision 45
# baseline (speedup 1.0000x reference)
"""AUTOGCN layer (2-hop GCN message passing + spectral filter mix) on 8 TRN2 NeuronCores.

Strategy (1D node-parallel, per sharding hint):
  - Nodes sharded 8 x 12500 by dst; each core owns the edges whose dst is in its shard.
  - Full feature table replicated in each core's HBM; per-edge gather via GPSIMD
    dma_gather (int16 indices -> 4 src ranges of 25000 rows).
  - Segment-sum over edges done on the TensorEngine: for each 128-edge chunk a
    selection matrix S[p, m] = w_e * (ldst[p] == m) is built on the VectorEngine
    (iota compare) and PSUM accumulates S^T @ G over all chunks of a 128-node tile.
  - w_e = norm[dst]*norm[src] (symmetric GCN normalization) folds both D^-1/2
    factors into the edge weight, so the gather tables are raw X and raw h.
  - Halo exchange: h shards AllGathered (4 tile-aligned chunks to let the
    collective start before stage 1 fully drains).
  - Stage 3 (dense): O_b = h @ (a_b W_b^T) + x @ (b_b W_b^T) per branch via PE,
    sigmoid cross-gating on ACT/DVE, out = relu(snorm * (sum + bias)).
"""

import sys, os
for _p in ("/opt/trn_rl_repo", "/root/.axon_site/_ro/trn_rl_repo"):
    if os.path.isdir(_p) and _p not in sys.path:
        sys.path.insert(0, _p)

import math
import numpy as np

# ---------------------------------------------------------------- constants
N = 100000
D = 256
CORES = 8
P = 128
EPS = 1e-9

_CACHE = {}


# ---------------------------------------------------------------- host prep
class Cfg:
    def __init__(self, n_nodes, d=D, cores=CORES, nr=None):
        assert n_nodes % cores == 0
        self.N = n_nodes
        self.D = d
        self.CORES = cores
        self.SHARD = n_nodes // cores
        self.TILES = (self.SHARD + P - 1) // P
        # stage-1 gather ranges over the full node table (int16 addressing)
        self.NR = nr if nr is not None else max(1, math.ceil(self.N / 32000))
        self.R1 = math.ceil(self.N / self.NR)
        # stage-2: split shard tiles into NR tile-aligned chunks; chunk q of
        # every core is allgathered into hfull_q = [CORES*rows_q, D]
        base = self.TILES // self.NR
        rem = self.TILES % self.NR
        self.CHUNK_TILES = [base + (1 if i < rem else 0) for i in range(self.NR)]
        assert all(ct * P * cores <= 32767 or ct == 0 for ct in self.CHUNK_TILES), \
            "stage-2 chunk exceeds int16 addressing"
        self.CHUNK_T0 = np.concatenate([[0], np.cumsum(self.CHUNK_TILES)]).astype(int)
        # shard-row boundaries of each chunk
        self.B2 = [min(t0 * P, self.SHARD) for t0 in self.CHUNK_T0]
        self.ROWS2 = [self.B2[q + 1] - self.B2[q] for q in range(self.NR)]

    def tile_rows(self, t):
        return min(P, self.SHARD - t * P)


def _wrap_idx(idx_padded):
    """[slots] int16 -> [128, slots/16] wrapped in 16 partitions, replicated x8."""
    blk16 = idx_padded.reshape(-1, 16).T  # [16, slots/16]
    return np.tile(blk16, (8, 1))  # [128, slots/16]


def _plan_stage(cfg, core_of_dst, t_of_dst, ldst_of_dst, r_of_src, idx_of_src, dst, src, w):
    """Build per-core flat arrays + shared meta for one mp stage.

    Returns (meta, idx_arrs, ldw_arrs):
      meta: list over tiles of dict(idx_off, ldw_off, Ct, groups=[(r, col_off, cols)])
      idx_arrs[c]: int16 flat array (concatenated per-tile [128, 8*Ct] blocks)
      ldst_arrs[c], w_arrs[c]: float32 flat arrays ([128, Ct] blocks per tile)
    """
    CORES_, TILES, NR = cfg.CORES, cfg.TILES, cfg.NR
    NG = TILES * NR
    gid = (core_of_dst * NG) + t_of_dst * NR + r_of_src  # [E]
    cnt = np.bincount(gid + 0, minlength=CORES_ * NG).reshape(CORES_, NG)
    cols_g = np.maximum(0, (cnt.max(axis=0) + P - 1) // P)  # [NG]

    # meta (same for all cores)
    meta = []
    idx_off = 0
    ldw_off = 0
    for t in range(TILES):
        groups = []
        col_off = 0
        for r in range(NR):
            c = int(cols_g[t * NR + r])
            if c > 0:
                groups.append((r, col_off, c))
                col_off += c
        Ct = col_off
        meta.append(dict(idx_off=idx_off, ldw_off=ldw_off, Ct=Ct, groups=groups))
        idx_off += 128 * 8 * Ct
        ldw_off += 128 * Ct
    tot_idx = idx_off
    tot_ldw = ldw_off

    slots_g = cols_g * P
    slot_base = np.concatenate([[0], np.cumsum(slots_g)]).astype(np.int64)  # per gid within a core
    total_slots = int(slot_base[-1])

    idx_arrs, ldst_arrs, w_arrs = [], [], []
    for c in range(CORES_):
        mask = core_of_dst == c
        g = gid[mask] - c * NG
        order = np.argsort(g, kind="stable")
        g_s = g[order]
        # rank within group
        n = len(g_s)
        starts = np.searchsorted(g_s, np.arange(NG))
        rank = np.arange(n) - starts[g_s]
        slot = slot_base[g_s] + rank

        idx_flat = np.zeros(total_slots, np.int16)     # pads gather row 0 (safe)
        ldst_flat = np.zeros(total_slots, np.float32)
        w_flat = np.zeros(total_slots, np.float32)
        idx_flat[slot] = idx_of_src[mask][order].astype(np.int16)
        ldst_flat[slot] = ldst_of_dst[mask][order].astype(np.float32)
        w_flat[slot] = w[mask][order].astype(np.float32)

        idx_out = np.zeros(tot_idx, np.int16)
        ldst_out = np.zeros(tot_ldw, np.float32)
        w_out = np.zeros(tot_ldw, np.float32)
        for t in range(TILES):
            m = meta[t]
            io, lo = m["idx_off"], m["ldw_off"]
            iblks, lblks, wblks = [], [], []
            for (r, coff, ncols) in m["groups"]:
                sb = slot_base[t * NR + r]
                sl = slice(sb, sb + ncols * P)
                iblks.append(_wrap_idx(idx_flat[sl]))
                lblks.append(ldst_flat[sl].reshape(ncols, P).T)
                wblks.append(w_flat[sl].reshape(ncols, P).T)
            if m["Ct"] == 0:
                continue
            idx_out[io:io + 128 * 8 * m["Ct"]] = np.concatenate(iblks, axis=1).ravel()
            ldst_out[lo:lo + 128 * m["Ct"]] = np.concatenate(lblks, axis=1).ravel()
            w_out[lo:lo + 128 * m["Ct"]] = np.concatenate(wblks, axis=1).ravel()
        idx_arrs.append(idx_out)
        ldst_arrs.append(ldst_out)
        w_arrs.append(w_out)
    return meta, idx_arrs, ldst_arrs, w_arrs


def plan_graph(cfg, src, dst):
    src = np.asarray(src).astype(np.int64).ravel()
    dst = np.asarray(dst).astype(np.int64).ravel()
    deg = np.bincount(dst, minlength=cfg.N).astype(np.float32)
    norm = np.clip(deg, 1.0, None) ** -0.5
    w = (norm[dst] * norm[src]).astype(np.float32)

    core_of = dst // cfg.SHARD
    loc = dst % cfg.SHARD
    t_of = loc // P
    ldst_of = loc % P

    # stage 1: gather from xfull, ranges of R1 rows
    r1 = src // cfg.R1
    i1 = src - r1 * cfg.R1
    meta1, idx1, ld1, w1 = _plan_stage(cfg, core_of, t_of, ldst_of, r1, i1, dst, src, w)

    # stage 2: gather from hfull_q (chunk-major layout)
    cs = src // cfg.SHARD
    j = src % cfg.SHARD
    q2 = np.digitize(j, cfg.B2[1:-1])  # 0..NR-1
    rows2 = np.asarray(cfg.ROWS2, np.int64)
    b2 = np.asarray(cfg.B2[:-1], np.int64)
    i2 = cs * rows2[q2] + (j - b2[q2])
    meta2, idx2, ld2, w2 = _plan_stage(cfg, core_of, t_of, ldst_of, q2, i2, dst, src, w)

    return dict(meta1=meta1, idx1=idx1, ld1=ld1, w1=w1,
                meta2=meta2, idx2=idx2, ld2=ld2, w2=w2, norm=norm)


def filter_scalars(low_gamma, mid_gamma, high_gamma, k):
    alpha = np.linspace(-EPS, 1.0 + EPS, k).astype(np.float32)
    gl = np.maximum(np.asarray(low_gamma, np.float32), 0.0)
    gm = np.maximum(np.asarray(mid_gamma, np.float32), 0.0)
    gh = np.maximum(np.asarray(high_gamma, np.float32), 0.0)
    a_l = float(alpha @ gl); b_l = float((1.0 - alpha) @ gl)
    a_h = float((-alpha) @ gh); b_h = float((1.0 - alpha) @ gh)
    s_m = float(gm.sum()); c_m = float(-(alpha @ gm))
    return a_l, b_l, a_h, b_h, s_m, c_m


# ---------------------------------------------------------------- program
def build_program(cfg, meta1, meta2, lens):
    import concourse.bass as bass
    import concourse.tile as tile
    from concourse import bacc, mybir

    f32 = mybir.dt.float32
    i16 = mybir.dt.int16
    bf16 = mybir.dt.bfloat16
    use_bf16 = os.environ.get("KERNEL_BF16", "0") == "1"
    gdt = bf16 if use_bf16 else f32  # dtype of gather tables / matmul operands
    Alu = mybir.AluOpType
    Act = mybir.ActivationFunctionType
    NR, TILES, SHARD, D_ = cfg.NR, cfg.TILES, cfg.SHARD, cfg.D

    nc = bacc.Bacc("TRN2", target_bir_lowering=False, debug=False,
                   num_devices=cfg.CORES)

    xfull = nc.dram_tensor("xfull", [cfg.N, D_], gdt, kind="ExternalInput")
    xshardT = nc.dram_tensor("xshardt", [P, 2, TILES * P], gdt, kind="ExternalInput")
    snorm = nc.dram_tensor("snorm", [SHARD, 1], f32, kind="ExternalInput")
    biasrep = nc.dram_tensor("biasrep", [P, D_], f32, kind="ExternalInput")
    wmats = nc.dram_tensor("wmats", [P, 12, D_], gdt, kind="ExternalInput")
    iota = nc.dram_tensor("iota", [P, P], f32, kind="ExternalInput")
    ident = nc.dram_tensor("ident", [P, P], f32, kind="ExternalInput")
    idx1 = nc.dram_tensor("idx1", [max(lens["idx1"], 16)], i16, kind="ExternalInput")
    ldst1 = nc.dram_tensor("ldst1", [max(lens["ldw1"], 4)], f32, kind="ExternalInput")
    w1 = nc.dram_tensor("w1", [max(lens["ldw1"], 4)], f32, kind="ExternalInput")
    idx2 = nc.dram_tensor("idx2", [max(lens["idx2"], 16)], i16, kind="ExternalInput")
    ldst2 = nc.dram_tensor("ldst2", [max(lens["ldw2"], 4)], f32, kind="ExternalInput")
    w2 = nc.dram_tensor("w2", [max(lens["ldw2"], 4)], f32, kind="ExternalInput")
    out = nc.dram_tensor("out", [SHARD, D_], f32, kind="ExternalOutput")

    hin = [nc.dram_tensor(f"hin{q}", [cfg.ROWS2[q], D_], gdt)
           for q in range(NR)]
    hfull = [nc.dram_tensor(f"hfull{q}", [cfg.CORES * cfg.ROWS2[q], D_], gdt,
                            addr_space="Shared") for q in range(NR)]
    replica_groups = [list(range(cfg.CORES))]

    max_cols = max(max((g[2] for g in m["groups"]), default=1)
                   for mm in (meta1, meta2) for m in mm)
    max_ct = max(m["Ct"] for mm in (meta1, meta2) for m in mm)

    with tile.TileContext(nc) as tc:
        with tc.tile_pool(name="consts", bufs=1) as cp, \
             tc.tile_pool(name="meta", bufs=6) as mp, \
             tc.tile_pool(name="gath", bufs=10) as gp, \
             tc.tile_pool(name="smat", bufs=8) as sp, \
             tc.tile_pool(name="hst", bufs=3) as hp, \
             tc.tile_pool(name="s3", bufs=3) as p3, \
             tc.tile_pool(name="mmacc", bufs=3, space="PSUM") as pacc, \
             tc.tile_pool(name="oacc", bufs=3, space="PSUM") as pout, \
             tc.tile_pool(name="tracc", bufs=2, space="PSUM") as ptr:

            iota_t = cp.tile([P, P], f32)
            nc.sync.dma_start(out=iota_t[:], in_=iota[:])
            ident_t = cp.tile([P, P], gdt)
            nc.gpsimd.dma_start(out=ident_t[:], in_=ident[:])  # SWDGE casts f32->gdt
            bias_t = cp.tile([P, D_], f32)
            nc.sync.dma_start(out=bias_t[:], in_=biasrep[:])
            wm_t = cp.tile([P, 12, D_], gdt)
            nc.sync.dma_start(out=wm_t[:], in_=wmats[:])

            probe = os.environ.get("KERNEL_PROBE", "")

            def mp_tile(t, stage, acc):
                """Emit gather + segment-sum for node tile t; returns #chunks."""
                meta = (meta1, meta2)[stage][t]
                Ct = meta["Ct"]
                if Ct == 0:
                    return 0
                idx_d, ld_d, w_d = (idx1, ldst1, w1) if stage == 0 else (idx2, ldst2, w2)
                io, lo = meta["idx_off"], meta["ldw_off"]
                # meta loads ride the ACT HWDGE ring so they don't contend
                # with the sync ring (h/x loads, h/out stores)
                idxt = mp.tile([P, 8 * max_ct], i16, tag="idx")
                nc.scalar.dma_start(
                    out=idxt[:, :8 * Ct],
                    in_=idx_d[io:io + 128 * 8 * Ct].rearrange("(p c) -> p c", p=P))
                ldt = mp.tile([P, max_ct], f32, tag="ld")
                nc.scalar.dma_start(
                    out=ldt[:, :Ct],
                    in_=ld_d[lo:lo + 128 * Ct].rearrange("(p c) -> p c", p=P))
                wt = mp.tile([P, max_ct], f32, tag="w")
                nc.scalar.dma_start(
                    out=wt[:, :Ct],
                    in_=w_d[lo:lo + 128 * Ct].rearrange("(p c) -> p c", p=P))

                gcap = int(os.environ.get("KERNEL_GCOLS", "2"))
                nq = int(os.environ.get("KERNEL_NQ", "1"))
                ci = 0
                for (r, coff, ncols) in meta["groups"]:
                    if stage == 0:
                        rlen = min(cfg.R1, cfg.N - r * cfg.R1)
                        table = xfull[r * cfg.R1: r * cfg.R1 + rlen, :]
                    else:
                        table = hfull[r][:, :]
                    # balanced split of ncols into ceil(ncols/gcap) calls
                    ncalls = -(-ncols // gcap)
                    base = ncols // ncalls
                    rem = ncols % ncalls
                    splits = [base + (1 if i < rem else 0) for i in range(ncalls)]
                    k0 = 0
                    for sub in splits:
                        slots = sub * P
                        g_t = gp.tile([P, min(gcap, max_cols), D_], gdt, tag="g")
                        nc.gpsimd.dma_gather(
                            out_ap=g_t[:, :sub, :],
                            in_ap=table,
                            idxs_ap=idxt[:, 8 * (coff + k0): 8 * (coff + k0 + sub)],
                            num_idxs=slots,
                            num_idxs_reg=slots,
                            elem_size=D_,
                            queue_num=(ci // max(1, gcap)) % nq,
                        )
                        if probe == "gather":
                            ci += sub
                            k0 += sub
                            continue
                        s_t = sp.tile([P, min(gcap, max_cols) * P], gdt, tag="s")
                        for k in range(sub):
                            nc.vector.tensor_scalar(
                                out=s_t[:, k * P:(k + 1) * P],
                                in0=iota_t[:],
                                scalar1=ldt[:, coff + k0 + k: coff + k0 + k + 1],
                                scalar2=wt[:, coff + k0 + k: coff + k0 + k + 1],
                                op0=Alu.is_equal,
                                op1=Alu.mult,
                            )
                        for k in range(sub):
                            nc.tensor.matmul(
                                out=acc[:],
                                lhsT=s_t[:, k * P:(k + 1) * P],
                                rhs=g_t[:, k, :],
                                start=(ci == 0),
                                stop=(ci == Ct - 1),
                            )
                            ci += 1
                        k0 += sub
                return Ct if probe != "gather" else 0

            # ---------------- stage 1: h = Dn A Dn X  -> hin chunks + allgather
            for q in range(NR):
                for t in range(cfg.CHUNK_T0[q], cfg.CHUNK_T0[q + 1]):
                    rows = cfg.tile_rows(t)
                    acc = pacc.tile([P, D_], f32, tag="acc")
                    nch = mp_tile(t, 0, acc)
                    ht = hp.tile([P, D_], gdt, tag="ht")
                    if nch == 0:
                        nc.vector.memset(ht[:], 0.0)
                    else:
                        nc.scalar.activation(out=ht[:], in_=acc[:], func=Act.Copy)
                    r0 = t * P - cfg.B2[q]
                    nc.sync.dma_start(out=hin[q][r0:r0 + rows, :], in_=ht[:rows, :])
                if cfg.ROWS2[q] > 0:
                    nc.gpsimd.collective_compute(
                        "AllGather",
                        mybir.AluOpType.bypass,
                        replica_groups=replica_groups,
                        ins=[hin[q][:]],
                        outs=[hfull[q][:]],
                    )

            # ---------------- stage 2 + fused stage 3 per tile
            for t in range(TILES):
                rows = cfg.tile_rows(t)
                q = int(np.digitize(t * P, cfg.B2[1:-1]))
                acc2 = pacc.tile([P, D_], f32, tag="acc")
                nch = mp_tile(t, 1, acc2)
                if probe == "gather":
                    ot0 = p3.tile([P, D_], f32, tag="ot")
                    nc.vector.memset(ot0[:], 0.0)
                    nc.sync.dma_start(out=out[t * P: t * P + rows, :], in_=ot0[:rows, :])
                    continue

                # h1 tile -> SBUF
                h1s = p3.tile([P, D_], gdt, tag="h1s")
                if nch == 0:
                    nc.vector.memset(h1s[:], 0.0)
                else:
                    nc.scalar.activation(out=h1s[:], in_=acc2[:], func=Act.Copy)
                # h tile from local hin
                hts = p3.tile([P, D_], gdt, tag="hts")
                if rows < P:
                    nc.vector.memset(hts[:], 0.0)
                r0 = t * P - cfg.B2[q]
                nc.sync.dma_start(out=hts[:rows, :], in_=hin[q][r0:r0 + rows, :])

                # x^T tile straight from the host-transposed layout
                xT = p3.tile([P, 2, P], gdt, tag="xT")
                nc.sync.dma_start(out=xT[:], in_=xshardT[:, :, t * P:(t + 1) * P])

                # transpose h and h1 (PE): hT/h1T [P, 2, P]
                hT = p3.tile([P, 2, P], gdt, tag="hT")
                h1T = p3.tile([P, 2, P], gdt, tag="h1T")
                for (src_t, dst_t) in ((hts, hT), (h1s, h1T)):
                    for j in range(2):
                        ptt = ptr.tile([P, P], gdt, tag="ptt")
                        nc.tensor.transpose(out=ptt[:], in_=src_t[:, j * P:(j + 1) * P],
                                            identity=ident_t[:])
                        nc.scalar.activation(out=dst_t[:, j, :], in_=ptt[:], func=Act.Copy)

                # branch matmuls: O = lhs1 @ w[a] + lhs2 @ w[b]
                # wmat slots: 0,1=a_l*WlT; 2,3=b_l*WlT; 4,5=a_h*WhT; 6,7=b_h*WhT;
                #             8,9=s_m*WmT; 10,11=c_m*WmT
                obuf = []
                for (hsrc, wa, wb) in ((hT, 0, 2), (hT, 4, 6), (h1T, 8, 10)):
                    o_ps = pout.tile([P, D_], f32, tag="ops")
                    for j in range(2):
                        nc.tensor.matmul(out=o_ps[:], lhsT=hsrc[:, j, :],
                                         rhs=wm_t[:, wa + j, :],
                                         start=(j == 0), stop=False)
                    for j in range(2):
                        nc.tensor.matmul(out=o_ps[:],
                                         lhsT=xT[:, j, :],
                                         rhs=wm_t[:, wb + j, :],
                                         start=False, stop=(j == 1))
                    o_sb = p3.tile([P, D_], f32, tag=f"o{len(obuf)}")
                    nc.scalar.activation(out=o_sb[:], in_=o_ps[:], func=Act.Copy)
                    obuf.append(o_sb)
                o_low, o_high, o_mid = obuf

                # sequential cross gating
                tmp = p3.tile([P, D_], f32, tag="tmp")
                sg = p3.tile([P, D_], f32, tag="sg")
                nc.vector.tensor_tensor(out=tmp[:], in0=o_high[:], in1=o_mid[:], op=Alu.add)
                nc.scalar.activation(out=sg[:], in_=tmp[:], func=Act.Sigmoid)
                nc.vector.tensor_tensor(out=o_low[:], in0=o_low[:], in1=sg[:], op=Alu.mult)
                nc.vector.tensor_tensor(out=tmp[:], in0=o_low[:], in1=o_high[:], op=Alu.add)
                nc.scalar.activation(out=sg[:], in_=tmp[:], func=Act.Sigmoid)
                nc.vector.tensor_tensor(out=o_mid[:], in0=o_mid[:], in1=sg[:], op=Alu.mult)
                nc.vector.tensor_tensor(out=tmp[:], in0=o_mid[:], in1=o_low[:], op=Alu.add)
                nc.scalar.activation(out=sg[:], in_=tmp[:], func=Act.Sigmoid)
                nc.vector.tensor_tensor(out=o_high[:], in0=o_high[:], in1=sg[:], op=Alu.mult)

                nc.vector.tensor_tensor(out=tmp[:], in0=o_low[:], in1=o_mid[:], op=Alu.add)
                nc.vector.tensor_tensor(out=tmp[:], in0=tmp[:], in1=o_high[:], op=Alu.add)
                nc.vector.tensor_tensor(out=tmp[:], in0=tmp[:], in1=bias_t[:], op=Alu.add)

                sn = p3.tile([P, 1], f32, tag="sn")
                if rows < P:
                    nc.vector.memset(sn[:], 0.0)
                nc.sync.dma_start(out=sn[:rows, :], in_=snorm[t * P: t * P + rows, :])
                ot = p3.tile([P, D_], f32, tag="ot")
                nc.scalar.activation(out=ot[:], in_=tmp[:], func=Act.Relu, scale=sn[:, :1])
                nc.sync.dma_start(out=out[t * P: t * P + rows, :], in_=ot[:rows, :])

    nc.compile()
    return nc


# ---------------------------------------------------------------- entry
def _build_in_maps(cfg, plan, feature, snorm_n, W_low, W_mid, W_high,
                   low_gamma, mid_gamma, high_gamma, bias, k):
    use_bf16 = os.environ.get("KERNEL_BF16", "0") == "1"
    if use_bf16:
        import ml_dtypes
        gnp = ml_dtypes.bfloat16
    else:
        gnp = np.float32
    a_l, b_l, a_h, b_h, s_m, c_m = filter_scalars(low_gamma, mid_gamma, high_gamma, k)
    wl = np.asarray(W_low, np.float32).T.copy()   # WlT[d, n]
    wh = np.asarray(W_high, np.float32).T.copy()
    wm = np.asarray(W_mid, np.float32).T.copy()
    wmats = np.zeros((P, 12, cfg.D), np.float32)
    for i, m in enumerate((a_l * wl, b_l * wl, a_h * wh, b_h * wh, s_m * wm, c_m * wm)):
        wmats[:, 2 * i, :] = m[:P, :]
        wmats[:, 2 * i + 1, :] = m[P:2 * P, :]
    wmats = wmats.astype(gnp)

    x = np.ascontiguousarray(np.asarray(feature, np.float32))
    sn = np.asarray(snorm_n, np.float32).reshape(cfg.N, 1)
    biasrep = np.tile(np.asarray(bias, np.float32).reshape(1, cfg.D), (P, 1))
    iota = np.tile(np.arange(P, dtype=np.float32), (P, 1))
    ident = np.eye(P, dtype=np.float32)

    xg = x.astype(gnp) if use_bf16 else x
    in_maps = []
    padsh = cfg.TILES * P
    for c in range(cfg.CORES):
        xs = x[c * cfg.SHARD:(c + 1) * cfg.SHARD]  # [SHARD, D]
        xsp = np.zeros((padsh, cfg.D), np.float32)
        xsp[:cfg.SHARD] = xs
        # xsT[p, j, col] = xs[col, j*128 + p]
        xsT = np.ascontiguousarray(
            xsp.T.reshape(2, P, padsh).transpose(1, 0, 2)).astype(gnp)  # [128, 2, padsh]
        in_maps.append(dict(
            xfull=xg,
            xshardt=xsT,
            snorm=np.ascontiguousarray(sn[c * cfg.SHARD:(c + 1) * cfg.SHARD]),
            biasrep=biasrep, wmats=wmats, iota=iota, ident=ident,
            idx1=_padlen(plan["idx1"][c], 16), ldst1=_padlen(plan["ld1"][c], 4),
            w1=_padlen(plan["w1"][c], 4),
            idx2=_padlen(plan["idx2"][c], 16), ldst2=_padlen(plan["ld2"][c], 4),
            w2=_padlen(plan["w2"][c], 4),
        ))
    return in_maps


def _padlen(a, mn):
    if len(a) >= mn:
        return a
    out = np.zeros(mn, a.dtype)
    out[:len(a)] = a
    return out


LAST_RESULT = None
LAST_RUNNER = None
LAST_EXEC_NS = None
LAST_FLOOR_NS = None
TIME_REPS = int(os.environ.get("KERNEL_TIME_REPS", "0"))


class _PjrtRunner:
    """Compile a Bass program once and run it across n cores via shard_map."""

    def __init__(self, nc, n_cores):
        import jax
        import numpy as _np
        from jax.sharding import Mesh, PartitionSpec, NamedSharding
        from jax.experimental.shard_map import shard_map
        from concourse import bass2jax, mybir

        bass2jax.install_neuronx_cc_hook()
        self.jax = jax
        self.n_cores = n_cores
        partition_name = (nc.partition_id_tensor.name
                          if nc.partition_id_tensor else None)
        in_names, out_names, out_avals, zero_outs = [], [], [], []
        for alloc in nc.m.functions[0].allocations:
            if not isinstance(alloc, mybir.MemoryLocationSet):
                continue
            name = alloc.memorylocations[0].name
            if alloc.kind == "ExternalInput":
                if name != partition_name:
                    in_names.append(name)
            elif alloc.kind == "ExternalOutput":
                out_names.append(name)
                shape = tuple(alloc.tensor_shape)
                dtype = mybir.dt.np(alloc.dtype)
                out_avals.append(jax.core.ShapedArray(shape, dtype))
                zero_outs.append(_np.zeros(shape, dtype))
        self.in_names, self.out_names = in_names, out_names
        self.zero_outs = zero_outs
        n_params = len(in_names)
        all_names = list(in_names + out_names)
        if partition_name is not None:
            all_names.append(partition_name)

        def _body(*args):
            operands = list(args)
            if partition_name is not None:
                operands.append(bass2jax.partition_id_tensor())
            outs = bass2jax._bass_exec_p.bind(
                *operands,
                out_avals=tuple(out_avals),
                in_names=tuple(all_names),
                out_names=tuple(out_names),
                lowering_input_output_aliases=(),
                sim_require_finite=True,
                sim_require_nnan=True,
                nc=nc,
            )
            return tuple(outs)

        devices = jax.devices()[:n_cores]
        self.mesh = Mesh(_np.asarray(devices), ("core",))
        self.sharding = NamedSharding(self.mesh, PartitionSpec("core"))
        donate = tuple(range(n_params, n_params + len(out_names)))
        self.fn = jax.jit(
            shard_map(_body, mesh=self.mesh,
                      in_specs=(PartitionSpec("core"),) * (n_params + len(out_names)),
                      out_specs=(PartitionSpec("core"),) * len(out_names),
                      check_rep=False),
            donate_argnums=donate, keep_unused=True)
        self.dev_in = None

    def put_inputs(self, in_maps):
        jax = self.jax
        concat = [np.concatenate([np.asarray(m[n]) for m in in_maps], axis=0)
                  for n in self.in_names]
        self.dev_in = [jax.device_put(a, self.sharding) for a in concat]

    def _zeros(self):
        jax = self.jax
        return [jax.device_put(
                    np.zeros((self.n_cores * z.shape[0], *z.shape[1:]), z.dtype),
                    self.sharding) for z in self.zero_outs]

    def execute(self):
        out = self.fn(*self.dev_in, *self._zeros())
        for o in out:
            o.block_until_ready()
        return out

    def timed(self, reps):
        import time as _t
        best = None
        for _ in range(reps):
            zs = self._zeros()
            for z in zs:
                z.block_until_ready()
            t0 = _t.perf_counter()
            out = self.fn(*self.dev_in, *zs)
            for o in out:
                o.block_until_ready()
            dt = _t.perf_counter() - t0
            best = dt if best is None else min(best, dt)
        return best

    def fetch(self, out):
        res = []
        for c in range(self.n_cores):
            d = {}
            for i, n in enumerate(self.out_names):
                full = np.asarray(out[i])
                d[n] = full.reshape(self.n_cores, full.shape[0] // self.n_cores,
                                    *full.shape[1:])[c]
            res.append(d)
        return res


def _floor_runner():
    """Tiny copy program to calibrate the axon dispatch floor."""
    if "floor" in _CACHE:
        return _CACHE["floor"]
    from concourse import bacc, mybir
    import concourse.tile as tile
    f32 = mybir.dt.float32
    nc = bacc.Bacc("TRN2", target_bir_lowering=False, debug=False, num_devices=CORES)
    a = nc.dram_tensor("a", [P, P], f32, kind="ExternalInput")
    b = nc.dram_tensor("b", [P, P], f32, kind="ExternalOutput")
    with tile.TileContext(nc) as tc:
        with tc.tile_pool(name="p", bufs=1) as pool:
            t = pool.tile([P, P], f32)
            nc.sync.dma_start(out=t[:], in_=a[:])
            nc.sync.dma_start(out=b[:], in_=t[:])
    nc.compile()
    r = _PjrtRunner(nc, CORES)
    r.put_inputs([dict(a=np.zeros((P, P), np.float32)) for _ in range(CORES)])
    r.execute()
    _CACHE["floor"] = r
    return r


def run(inputs, cfg=None):
    global LAST_EXEC_NS, LAST_FLOOR_NS
    os.environ.setdefault("KERNEL_BF16", "1")
    if cfg is None:
        cfg = Cfg(N)
    feature = np.asarray(inputs["feature"], np.float32)
    k = int(np.asarray(inputs["low_gamma"]).shape[0])

    plan = plan_graph(cfg, inputs["src"], inputs["dst"])
    lens = dict(idx1=len(plan["idx1"][0]), ldw1=len(plan["ld1"][0]),
                idx2=len(plan["idx2"][0]), ldw2=len(plan["ld2"][0]))

    key = (cfg.N, cfg.D, lens["idx1"], lens["ldw1"], lens["idx2"], lens["ldw2"],
           os.environ.get("KERNEL_BF16", "0"), os.environ.get("KERNEL_GCOLS", "2"),
           os.environ.get("KERNEL_PROBE", ""),
           tuple(m["idx_off"] for m in plan["meta1"]),
           tuple(m["idx_off"] for m in plan["meta2"]))
    if key not in _CACHE:
        nc = build_program(cfg, plan["meta1"], plan["meta2"], lens)
        _CACHE[key] = _PjrtRunner(nc, cfg.CORES)
    runner = _CACHE[key]

    in_maps = _build_in_maps(cfg, plan, feature, inputs["snorm_n"],
                             inputs["W_low"], inputs["W_mid"], inputs["W_high"],
                             inputs["low_gamma"], inputs["mid_gamma"],
                             inputs["high_gamma"], inputs["bias"], k)
    global LAST_RUNNER
    LAST_RUNNER = runner
    runner.put_inputs(in_maps)
    out_arrs = runner.execute()
    if TIME_REPS > 0:
        t_k = runner.timed(TIME_REPS)
        fl = _floor_runner()
        t_f = fl.timed(TIME_REPS)
        LAST_EXEC_NS = int((t_k - t_f) * 1e9)
        LAST_FLOOR_NS = int(t_f * 1e9)
    res = runner.fetch(out_arrs)
    out = np.concatenate([res[c]["out"] for c in range(cfg.CORES)], axis=0)
    return out


def kernel(**inputs):
    return run(inputs)


# revision 46
# speedup vs baseline: 1.1204x; 1.1204x over previous
"""AUTOGCN layer (2-hop GCN message passing + spectral filter mix) on 8 TRN2 NeuronCores.

Strategy (1D node-parallel, per sharding hint):
  - Nodes sharded 8 x 12500 by dst; each core owns the edges whose dst is in its shard.
  - Full feature table replicated in each core's HBM; per-edge gather via GPSIMD
    dma_gather (int16 indices -> 4 src ranges of 25000 rows).
  - Segment-sum over edges done on the TensorEngine: for each 128-edge chunk a
    selection matrix S[p, m] = w_e * (ldst[p] == m) is built on the VectorEngine
    (iota compare) and PSUM accumulates S^T @ G over all chunks of a 128-node tile.
  - w_e = norm[dst]*norm[src] (symmetric GCN normalization) folds both D^-1/2
    factors into the edge weight, so the gather tables are raw X and raw h.
  - Halo exchange: h shards AllGathered (4 tile-aligned chunks to let the
    collective start before stage 1 fully drains).
  - Stage 3 (dense): O_b = h @ (a_b W_b^T) + x @ (b_b W_b^T) per branch via PE,
    sigmoid cross-gating on ACT/DVE, out = relu(snorm * (sum + bias)).
"""

import sys, os
for _p in ("/opt/trn_rl_repo", "/root/.axon_site/_ro/trn_rl_repo"):
    if os.path.isdir(_p) and _p not in sys.path:
        sys.path.insert(0, _p)

import math
import numpy as np

# ---------------------------------------------------------------- constants
N = 100000
D = 256
CORES = 8
P = 128
EPS = 1e-9

_CACHE = {}


# ---------------------------------------------------------------- host prep
class Cfg:
    def __init__(self, n_nodes, d=D, cores=CORES, nr=None):
        assert n_nodes % cores == 0
        self.N = n_nodes
        self.D = d
        self.CORES = cores
        self.SHARD = n_nodes // cores
        self.TILES = (self.SHARD + P - 1) // P
        # stage-1 gather ranges over the full node table (int16 addressing)
        self.NR = nr if nr is not None else max(1, math.ceil(self.N / 32000))
        self.R1 = math.ceil(self.N / self.NR)
        # stage-2: split shard tiles into NR tile-aligned chunks; chunk q of
        # every core is allgathered into hfull_q = [CORES*rows_q, D]
        base = self.TILES // self.NR
        rem = self.TILES % self.NR
        self.CHUNK_TILES = [base + (1 if i < rem else 0) for i in range(self.NR)]
        assert all(ct * P * cores <= 32767 or ct == 0 for ct in self.CHUNK_TILES), \
            "stage-2 chunk exceeds int16 addressing"
        self.CHUNK_T0 = np.concatenate([[0], np.cumsum(self.CHUNK_TILES)]).astype(int)
        # shard-row boundaries of each chunk
        self.B2 = [min(t0 * P, self.SHARD) for t0 in self.CHUNK_T0]
        self.ROWS2 = [self.B2[q + 1] - self.B2[q] for q in range(self.NR)]

    def tile_rows(self, t):
        return min(P, self.SHARD - t * P)


def _wrap_idx(idx_padded):
    """[slots] int16 -> [128, slots/16] wrapped in 16 partitions, replicated x8."""
    blk16 = idx_padded.reshape(-1, 16).T  # [16, slots/16]
    return np.tile(blk16, (8, 1))  # [128, slots/16]


def _plan_stage(cfg, core_of_dst, t_of_dst, ldst_of_dst, r_of_src, idx_of_src, dst, src, w):
    """Build per-core flat arrays + shared meta for one mp stage.

    Returns (meta, idx_arrs, ldw_arrs):
      meta: list over tiles of dict(idx_off, ldw_off, Ct, groups=[(r, col_off, cols)])
      idx_arrs[c]: int16 flat array (concatenated per-tile [128, 8*Ct] blocks)
      ldst_arrs[c], w_arrs[c]: float32 flat arrays ([128, Ct] blocks per tile)
    """
    CORES_, TILES, NR = cfg.CORES, cfg.TILES, cfg.NR
    NG = TILES * NR
    gid = (core_of_dst * NG) + t_of_dst * NR + r_of_src  # [E]
    cnt = np.bincount(gid + 0, minlength=CORES_ * NG).reshape(CORES_, NG)
    cols_g = np.maximum(0, (cnt.max(axis=0) + P - 1) // P)  # [NG]

    # meta (same for all cores)
    meta = []
    idx_off = 0
    ldw_off = 0
    for t in range(TILES):
        groups = []
        col_off = 0
        for r in range(NR):
            c = int(cols_g[t * NR + r])
            if c > 0:
                groups.append((r, col_off, c))
                col_off += c
        Ct = col_off
        meta.append(dict(idx_off=idx_off, ldw_off=ldw_off, Ct=Ct, groups=groups))
        idx_off += 128 * 8 * Ct
        ldw_off += 128 * Ct
    tot_idx = idx_off
    tot_ldw = ldw_off

    slots_g = cols_g * P
    slot_base = np.concatenate([[0], np.cumsum(slots_g)]).astype(np.int64)  # per gid within a core
    total_slots = int(slot_base[-1])

    idx_arrs, ldst_arrs, w_arrs = [], [], []
    for c in range(CORES_):
        mask = core_of_dst == c
        g = gid[mask] - c * NG
        order = np.argsort(g, kind="stable")
        g_s = g[order]
        # rank within group
        n = len(g_s)
        starts = np.searchsorted(g_s, np.arange(NG))
        rank = np.arange(n) - starts[g_s]
        slot = slot_base[g_s] + rank

        idx_flat = np.zeros(total_slots, np.int16)     # pads gather row 0 (safe)
        ldst_flat = np.zeros(total_slots, np.float32)
        w_flat = np.zeros(total_slots, np.float32)
        idx_flat[slot] = idx_of_src[mask][order].astype(np.int16)
        ldst_flat[slot] = ldst_of_dst[mask][order].astype(np.float32)
        w_flat[slot] = w[mask][order].astype(np.float32)

        idx_out = np.zeros(tot_idx, np.int16)
        ldst_out = np.zeros(tot_ldw, np.float32)
        w_out = np.zeros(tot_ldw, np.float32)
        for t in range(TILES):
            m = meta[t]
            io, lo = m["idx_off"], m["ldw_off"]
            iblks, lblks, wblks = [], [], []
            for (r, coff, ncols) in m["groups"]:
                sb = slot_base[t * NR + r]
                sl = slice(sb, sb + ncols * P)
                iblks.append(_wrap_idx(idx_flat[sl]))
                lblks.append(ldst_flat[sl].reshape(ncols, P).T)
                wblks.append(w_flat[sl].reshape(ncols, P).T)
            if m["Ct"] == 0:
                continue
            idx_out[io:io + 128 * 8 * m["Ct"]] = np.concatenate(iblks, axis=1).ravel()
            ldst_out[lo:lo + 128 * m["Ct"]] = np.concatenate(lblks, axis=1).ravel()
            w_out[lo:lo + 128 * m["Ct"]] = np.concatenate(wblks, axis=1).ravel()
        idx_arrs.append(idx_out)
        ldst_arrs.append(ldst_out)
        w_arrs.append(w_out)
    return meta, idx_arrs, ldst_arrs, w_arrs


def plan_graph(cfg, src, dst):
    src = np.asarray(src).astype(np.int64).ravel()
    dst = np.asarray(dst).astype(np.int64).ravel()
    deg = np.bincount(dst, minlength=cfg.N).astype(np.float32)
    norm = np.clip(deg, 1.0, None) ** -0.5
    w = (norm[dst] * norm[src]).astype(np.float32)

    core_of = dst // cfg.SHARD
    loc = dst % cfg.SHARD
    t_of = loc // P
    ldst_of = loc % P

    # stage 1: gather from xfull, ranges of R1 rows
    r1 = src // cfg.R1
    i1 = src - r1 * cfg.R1
    meta1, idx1, ld1, w1 = _plan_stage(cfg, core_of, t_of, ldst_of, r1, i1, dst, src, w)

    # stage 2: gather from hfull_q (chunk-major layout)
    cs = src // cfg.SHARD
    j = src % cfg.SHARD
    q2 = np.digitize(j, cfg.B2[1:-1])  # 0..NR-1
    rows2 = np.asarray(cfg.ROWS2, np.int64)
    b2 = np.asarray(cfg.B2[:-1], np.int64)
    i2 = cs * rows2[q2] + (j - b2[q2])
    meta2, idx2, ld2, w2 = _plan_stage(cfg, core_of, t_of, ldst_of, q2, i2, dst, src, w)

    return dict(meta1=meta1, idx1=idx1, ld1=ld1, w1=w1,
                meta2=meta2, idx2=idx2, ld2=ld2, w2=w2, norm=norm)


def filter_scalars(low_gamma, mid_gamma, high_gamma, k):
    alpha = np.linspace(-EPS, 1.0 + EPS, k).astype(np.float32)
    gl = np.maximum(np.asarray(low_gamma, np.float32), 0.0)
    gm = np.maximum(np.asarray(mid_gamma, np.float32), 0.0)
    gh = np.maximum(np.asarray(high_gamma, np.float32), 0.0)
    a_l = float(alpha @ gl); b_l = float((1.0 - alpha) @ gl)
    a_h = float((-alpha) @ gh); b_h = float((1.0 - alpha) @ gh)
    s_m = float(gm.sum()); c_m = float(-(alpha @ gm))
    return a_l, b_l, a_h, b_h, s_m, c_m


# ---------------------------------------------------------------- program
def build_program(cfg, meta1, meta2, lens):
    import concourse.bass as bass
    import concourse.tile as tile
    from concourse import bacc, mybir

    f32 = mybir.dt.float32
    i16 = mybir.dt.int16
    bf16 = mybir.dt.bfloat16
    use_bf16 = os.environ.get("KERNEL_BF16", "0") == "1"
    gdt = bf16 if use_bf16 else f32  # dtype of gather tables / matmul operands
    Alu = mybir.AluOpType
    Act = mybir.ActivationFunctionType
    NR, TILES, SHARD, D_ = cfg.NR, cfg.TILES, cfg.SHARD, cfg.D

    nc = bacc.Bacc("TRN2", target_bir_lowering=False, debug=False,
                   num_devices=cfg.CORES)

    xfull = nc.dram_tensor("xfull", [cfg.N, D_], gdt, kind="ExternalInput")
    xshardT = nc.dram_tensor("xshardt", [P, 2, TILES * P], gdt, kind="ExternalInput")
    snorm = nc.dram_tensor("snorm", [SHARD, 1], f32, kind="ExternalInput")
    biasrep = nc.dram_tensor("biasrep", [P, D_], f32, kind="ExternalInput")
    wmats = nc.dram_tensor("wmats", [P, 12, D_], gdt, kind="ExternalInput")
    iota = nc.dram_tensor("iota", [P, P], f32, kind="ExternalInput")
    ident = nc.dram_tensor("ident", [P, P], f32, kind="ExternalInput")
    idx1 = nc.dram_tensor("idx1", [max(lens["idx1"], 16)], i16, kind="ExternalInput")
    ldst1 = nc.dram_tensor("ldst1", [max(lens["ldw1"], 4)], f32, kind="ExternalInput")
    w1 = nc.dram_tensor("w1", [max(lens["ldw1"], 4)], f32, kind="ExternalInput")
    idx2 = nc.dram_tensor("idx2", [max(lens["idx2"], 16)], i16, kind="ExternalInput")
    ldst2 = nc.dram_tensor("ldst2", [max(lens["ldw2"], 4)], f32, kind="ExternalInput")
    w2 = nc.dram_tensor("w2", [max(lens["ldw2"], 4)], f32, kind="ExternalInput")
    out = nc.dram_tensor("out", [SHARD, D_], f32, kind="ExternalOutput")

    hin = [nc.dram_tensor(f"hin{q}", [cfg.ROWS2[q], D_], gdt)
           for q in range(NR)]
    hfull = [nc.dram_tensor(f"hfull{q}", [cfg.CORES * cfg.ROWS2[q], D_], gdt,
                            addr_space="Shared") for q in range(NR)]
    replica_groups = [list(range(cfg.CORES))]

    max_cols = max(max((g[2] for g in m["groups"]), default=1)
                   for mm in (meta1, meta2) for m in mm)
    max_ct = max(m["Ct"] for mm in (meta1, meta2) for m in mm)

    with tile.TileContext(nc) as tc:
        with tc.tile_pool(name="consts", bufs=1) as cp, \
             tc.tile_pool(name="meta", bufs=6) as mp, \
             tc.tile_pool(name="gath", bufs=10) as gp, \
             tc.tile_pool(name="smat", bufs=8) as sp, \
             tc.tile_pool(name="hst", bufs=3) as hp, \
             tc.tile_pool(name="s3", bufs=3) as p3, \
             tc.tile_pool(name="mmacc", bufs=3, space="PSUM") as pacc, \
             tc.tile_pool(name="oacc", bufs=3, space="PSUM") as pout, \
             tc.tile_pool(name="tracc", bufs=2, space="PSUM") as ptr:

            iota_t = cp.tile([P, P], f32)
            nc.sync.dma_start(out=iota_t[:], in_=iota[:])
            ident_t = cp.tile([P, P], gdt)
            nc.gpsimd.dma_start(out=ident_t[:], in_=ident[:])  # SWDGE casts f32->gdt
            bias_t = cp.tile([P, D_], f32)
            nc.sync.dma_start(out=bias_t[:], in_=biasrep[:])
            wm_t = cp.tile([P, 12, D_], gdt)
            nc.sync.dma_start(out=wm_t[:], in_=wmats[:])

            probe = os.environ.get("KERNEL_PROBE", "")

            def mp_tile(t, stage, acc):
                """Emit gather + segment-sum for node tile t; returns #chunks."""
                meta = (meta1, meta2)[stage][t]
                Ct = meta["Ct"]
                if Ct == 0:
                    return 0
                idx_d, ld_d, w_d = (idx1, ldst1, w1) if stage == 0 else (idx2, ldst2, w2)
                io, lo = meta["idx_off"], meta["ldw_off"]
                # meta loads ride the ACT HWDGE ring so they don't contend
                # with the sync ring (h/x loads, h/out stores)
                idxt = mp.tile([P, 8 * max_ct], i16, tag="idx")
                nc.scalar.dma_start(
                    out=idxt[:, :8 * Ct],
                    in_=idx_d[io:io + 128 * 8 * Ct].rearrange("(p c) -> p c", p=P))
                ldt = mp.tile([P, max_ct], f32, tag="ld")
                nc.scalar.dma_start(
                    out=ldt[:, :Ct],
                    in_=ld_d[lo:lo + 128 * Ct].rearrange("(p c) -> p c", p=P))
                wt = mp.tile([P, max_ct], f32, tag="w")
                nc.scalar.dma_start(
                    out=wt[:, :Ct],
                    in_=w_d[lo:lo + 128 * Ct].rearrange("(p c) -> p c", p=P))

                gcap = int(os.environ.get("KERNEL_GCOLS", "2"))
                nq = int(os.environ.get("KERNEL_NQ", "1"))
                ci = 0
                for (r, coff, ncols) in meta["groups"]:
                    if stage == 0:
                        rlen = min(cfg.R1, cfg.N - r * cfg.R1)
                        table = xfull[r * cfg.R1: r * cfg.R1 + rlen, :]
                    else:
                        table = hfull[r][:, :]
                    # balanced split of ncols into ceil(ncols/gcap) calls
                    ncalls = -(-ncols // gcap)
                    base = ncols // ncalls
                    rem = ncols % ncalls
                    splits = [base + (1 if i < rem else 0) for i in range(ncalls)]
                    k0 = 0
                    for sub in splits:
                        slots = sub * P
                        g_t = gp.tile([P, min(gcap, max_cols), D_], gdt, tag="g")
                        nc.gpsimd.dma_gather(
                            out_ap=g_t[:, :sub, :],
                            in_ap=table,
                            idxs_ap=idxt[:, 8 * (coff + k0): 8 * (coff + k0 + sub)],
                            num_idxs=slots,
                            num_idxs_reg=slots,
                            elem_size=D_,
                            queue_num=(ci // max(1, gcap)) % nq,
                        )
                        if probe == "gather":
                            ci += sub
                            k0 += sub
                            continue
                        s_t = sp.tile([P, min(gcap, max_cols) * P], gdt, tag="s")
                        for k in range(sub):
                            nc.vector.tensor_scalar(
                                out=s_t[:, k * P:(k + 1) * P],
                                in0=iota_t[:],
                                scalar1=ldt[:, coff + k0 + k: coff + k0 + k + 1],
                                scalar2=wt[:, coff + k0 + k: coff + k0 + k + 1],
                                op0=Alu.is_equal,
                                op1=Alu.mult,
                            )
                        for k in range(sub):
                            nc.tensor.matmul(
                                out=acc[:],
                                lhsT=s_t[:, k * P:(k + 1) * P],
                                rhs=g_t[:, k, :],
                                start=(ci == 0),
                                stop=(ci == Ct - 1),
                            )
                            ci += 1
                        k0 += sub
                return Ct if probe != "gather" else 0

            # ---------------- stage 1: h = Dn A Dn X  -> hin chunks + allgather
            for q in range(NR):
                for t in range(cfg.CHUNK_T0[q], cfg.CHUNK_T0[q + 1]):
                    rows = cfg.tile_rows(t)
                    acc = pacc.tile([P, D_], f32, tag="acc")
                    nch = mp_tile(t, 0, acc)
                    ht = hp.tile([P, D_], gdt, tag="ht")
                    if nch == 0:
                        nc.vector.memset(ht[:], 0.0)
                    else:
                        nc.scalar.activation(out=ht[:], in_=acc[:], func=Act.Copy)
                    r0 = t * P - cfg.B2[q]
                    nc.sync.dma_start(out=hin[q][r0:r0 + rows, :], in_=ht[:rows, :])
                if cfg.ROWS2[q] > 0:
                    nc.gpsimd.collective_compute(
                        "AllGather",
                        mybir.AluOpType.bypass,
                        replica_groups=replica_groups,
                        ins=[hin[q][:]],
                        outs=[hfull[q][:]],
                    )

            # ---------------- stage 2 + fused stage 3 per tile
            for t in range(TILES):
                rows = cfg.tile_rows(t)
                q = int(np.digitize(t * P, cfg.B2[1:-1]))
                acc2 = pacc.tile([P, D_], f32, tag="acc")
                nch = mp_tile(t, 1, acc2)
                if probe == "gather":
                    ot0 = p3.tile([P, D_], f32, tag="ot")
                    nc.vector.memset(ot0[:], 0.0)
                    nc.sync.dma_start(out=out[t * P: t * P + rows, :], in_=ot0[:rows, :])
                    continue

                # h1 tile -> SBUF
                h1s = p3.tile([P, D_], gdt, tag="h1s")
                if nch == 0:
                    nc.vector.memset(h1s[:], 0.0)
                else:
                    nc.scalar.activation(out=h1s[:], in_=acc2[:], func=Act.Copy)
                # h tile from local hin
                hts = p3.tile([P, D_], gdt, tag="hts")
                if rows < P:
                    nc.vector.memset(hts[:], 0.0)
                r0 = t * P - cfg.B2[q]
                nc.sync.dma_start(out=hts[:rows, :], in_=hin[q][r0:r0 + rows, :])

                # x^T tile straight from the host-transposed layout
                xT = p3.tile([P, 2, P], gdt, tag="xT")
                nc.sync.dma_start(out=xT[:], in_=xshardT[:, :, t * P:(t + 1) * P])

                # transpose h and h1 (PE): hT/h1T [P, 2, P]
                hT = p3.tile([P, 2, P], gdt, tag="hT")
                h1T = p3.tile([P, 2, P], gdt, tag="h1T")
                for (src_t, dst_t) in ((hts, hT), (h1s, h1T)):
                    for j in range(2):
                        ptt = ptr.tile([P, P], gdt, tag="ptt")
                        nc.tensor.transpose(out=ptt[:], in_=src_t[:, j * P:(j + 1) * P],
                                            identity=ident_t[:])
                        nc.scalar.activation(out=dst_t[:, j, :], in_=ptt[:], func=Act.Copy)

                # branch matmuls: O = lhs1 @ w[a] + lhs2 @ w[b]
                # wmat slots: 0,1=a_l*WlT; 2,3=b_l*WlT; 4,5=a_h*WhT; 6,7=b_h*WhT;
                #             8,9=s_m*WmT; 10,11=c_m*WmT
                obuf = []
                for (hsrc, wa, wb) in ((hT, 0, 2), (hT, 4, 6), (h1T, 8, 10)):
                    o_ps = pout.tile([P, D_], f32, tag="ops")
                    for j in range(2):
                        nc.tensor.matmul(out=o_ps[:], lhsT=hsrc[:, j, :],
                                         rhs=wm_t[:, wa + j, :],
                                         start=(j == 0), stop=False)
                    for j in range(2):
                        nc.tensor.matmul(out=o_ps[:],
                                         lhsT=xT[:, j, :],
                                         rhs=wm_t[:, wb + j, :],
                                         start=False, stop=(j == 1))
                    o_sb = p3.tile([P, D_], f32, tag=f"o{len(obuf)}")
                    nc.scalar.activation(out=o_sb[:], in_=o_ps[:], func=Act.Copy)
                    obuf.append(o_sb)
                o_low, o_high, o_mid = obuf

                # sequential cross gating
                tmp = p3.tile([P, D_], f32, tag="tmp")
                sg = p3.tile([P, D_], f32, tag="sg")
                nc.vector.tensor_tensor(out=tmp[:], in0=o_high[:], in1=o_mid[:], op=Alu.add)
                nc.scalar.activation(out=sg[:], in_=tmp[:], func=Act.Sigmoid)
                nc.vector.tensor_tensor(out=o_low[:], in0=o_low[:], in1=sg[:], op=Alu.mult)
                nc.vector.tensor_tensor(out=tmp[:], in0=o_low[:], in1=o_high[:], op=Alu.add)
                nc.scalar.activation(out=sg[:], in_=tmp[:], func=Act.Sigmoid)
                nc.vector.tensor_tensor(out=o_mid[:], in0=o_mid[:], in1=sg[:], op=Alu.mult)
                nc.vector.tensor_tensor(out=tmp[:], in0=o_mid[:], in1=o_low[:], op=Alu.add)
                nc.scalar.activation(out=sg[:], in_=tmp[:], func=Act.Sigmoid)
                nc.vector.tensor_tensor(out=o_high[:], in0=o_high[:], in1=sg[:], op=Alu.mult)

                nc.vector.tensor_tensor(out=tmp[:], in0=o_low[:], in1=o_mid[:], op=Alu.add)
                nc.vector.tensor_tensor(out=tmp[:], in0=tmp[:], in1=o_high[:], op=Alu.add)
                nc.vector.tensor_tensor(out=tmp[:], in0=tmp[:], in1=bias_t[:], op=Alu.add)

                sn = p3.tile([P, 1], f32, tag="sn")
                if rows < P:
                    nc.vector.memset(sn[:], 0.0)
                nc.sync.dma_start(out=sn[:rows, :], in_=snorm[t * P: t * P + rows, :])
                ot = p3.tile([P, D_], f32, tag="ot")
                nc.scalar.activation(out=ot[:], in_=tmp[:], func=Act.Relu, scale=sn[:, :1])
                nc.sync.dma_start(out=out[t * P: t * P + rows, :], in_=ot[:rows, :])

    nc.compile()
    return nc


# ---------------------------------------------------------------- entry
def _build_in_maps(cfg, plan, feature, snorm_n, W_low, W_mid, W_high,
                   low_gamma, mid_gamma, high_gamma, bias, k):
    use_bf16 = os.environ.get("KERNEL_BF16", "0") == "1"
    if use_bf16:
        import ml_dtypes
        gnp = ml_dtypes.bfloat16
    else:
        gnp = np.float32
    a_l, b_l, a_h, b_h, s_m, c_m = filter_scalars(low_gamma, mid_gamma, high_gamma, k)
    wl = np.asarray(W_low, np.float32).T.copy()   # WlT[d, n]
    wh = np.asarray(W_high, np.float32).T.copy()
    wm = np.asarray(W_mid, np.float32).T.copy()
    wmats = np.zeros((P, 12, cfg.D), np.float32)
    for i, m in enumerate((a_l * wl, b_l * wl, a_h * wh, b_h * wh, s_m * wm, c_m * wm)):
        wmats[:, 2 * i, :] = m[:P, :]
        wmats[:, 2 * i + 1, :] = m[P:2 * P, :]
    wmats = wmats.astype(gnp)

    x = np.ascontiguousarray(np.asarray(feature, np.float32))
    sn = np.asarray(snorm_n, np.float32).reshape(cfg.N, 1)
    biasrep = np.tile(np.asarray(bias, np.float32).reshape(1, cfg.D), (P, 1))
    iota = np.tile(np.arange(P, dtype=np.float32), (P, 1))
    ident = np.eye(P, dtype=np.float32)

    xg = x.astype(gnp) if use_bf16 else x
    in_maps = []
    padsh = cfg.TILES * P
    for c in range(cfg.CORES):
        xs = x[c * cfg.SHARD:(c + 1) * cfg.SHARD]  # [SHARD, D]
        xsp = np.zeros((padsh, cfg.D), np.float32)
        xsp[:cfg.SHARD] = xs
        # xsT[p, j, col] = xs[col, j*128 + p]
        xsT = np.ascontiguousarray(
            xsp.T.reshape(2, P, padsh).transpose(1, 0, 2)).astype(gnp)  # [128, 2, padsh]
        in_maps.append(dict(
            xfull=xg,
            xshardt=xsT,
            snorm=np.ascontiguousarray(sn[c * cfg.SHARD:(c + 1) * cfg.SHARD]),
            biasrep=biasrep, wmats=wmats, iota=iota, ident=ident,
            idx1=_padlen(plan["idx1"][c], 16), ldst1=_padlen(plan["ld1"][c], 4),
            w1=_padlen(plan["w1"][c], 4),
            idx2=_padlen(plan["idx2"][c], 16), ldst2=_padlen(plan["ld2"][c], 4),
            w2=_padlen(plan["w2"][c], 4),
        ))
    return in_maps


def _padlen(a, mn):
    if len(a) >= mn:
        return a
    out = np.zeros(mn, a.dtype)
    out[:len(a)] = a
    return out


LAST_RESULT = None
LAST_RUNNER = None
LAST_EXEC_NS = None
LAST_FLOOR_NS = None
TIME_REPS = int(os.environ.get("KERNEL_TIME_REPS", "0"))


class _PjrtRunner:
    """Compile a Bass program once and run it across n cores via shard_map."""

    def __init__(self, nc, n_cores):
        import jax
        import numpy as _np
        from jax.sharding import Mesh, PartitionSpec, NamedSharding
        from jax.experimental.shard_map import shard_map
        from concourse import bass2jax, mybir

        bass2jax.install_neuronx_cc_hook()
        self.jax = jax
        self.n_cores = n_cores
        partition_name = (nc.partition_id_tensor.name
                          if nc.partition_id_tensor else None)
        in_names, out_names, out_avals, zero_outs = [], [], [], []
        for alloc in nc.m.functions[0].allocations:
            if not isinstance(alloc, mybir.MemoryLocationSet):
                continue
            name = alloc.memorylocations[0].name
            if alloc.kind == "ExternalInput":
                if name != partition_name:
                    in_names.append(name)
            elif alloc.kind == "ExternalOutput":
                out_names.append(name)
                shape = tuple(alloc.tensor_shape)
                dtype = mybir.dt.np(alloc.dtype)
                out_avals.append(jax.core.ShapedArray(shape, dtype))
                zero_outs.append(_np.zeros(shape, dtype))
        self.in_names, self.out_names = in_names, out_names
        self.zero_outs = zero_outs
        n_params = len(in_names)
        all_names = list(in_names + out_names)
        if partition_name is not None:
            all_names.append(partition_name)

        def _body(*args):
            operands = list(args)
            if partition_name is not None:
                operands.append(bass2jax.partition_id_tensor())
            outs = bass2jax._bass_exec_p.bind(
                *operands,
                out_avals=tuple(out_avals),
                in_names=tuple(all_names),
                out_names=tuple(out_names),
                lowering_input_output_aliases=(),
                sim_require_finite=True,
                sim_require_nnan=True,
                nc=nc,
            )
            return tuple(outs)

        devices = jax.devices()[:n_cores]
        self.mesh = Mesh(_np.asarray(devices), ("core",))
        self.sharding = NamedSharding(self.mesh, PartitionSpec("core"))
        donate = tuple(range(n_params, n_params + len(out_names)))
        self.fn = jax.jit(
            shard_map(_body, mesh=self.mesh,
                      in_specs=(PartitionSpec("core"),) * (n_params + len(out_names)),
                      out_specs=(PartitionSpec("core"),) * len(out_names),
                      check_rep=False),
            donate_argnums=donate, keep_unused=True)
        self.dev_in = None

    def put_inputs(self, in_maps):
        jax = self.jax
        concat = [np.concatenate([np.asarray(m[n]) for m in in_maps], axis=0)
                  for n in self.in_names]
        self.dev_in = [jax.device_put(a, self.sharding) for a in concat]

    def _zeros(self):
        jax = self.jax
        return [jax.device_put(
                    np.zeros((self.n_cores * z.shape[0], *z.shape[1:]), z.dtype),
                    self.sharding) for z in self.zero_outs]

    def execute(self):
        out = self.fn(*self.dev_in, *self._zeros())
        for o in out:
            o.block_until_ready()
        return out

    def timed(self, reps):
        import time as _t
        best = None
        for _ in range(reps):
            zs = self._zeros()
            for z in zs:
                z.block_until_ready()
            t0 = _t.perf_counter()
            out = self.fn(*self.dev_in, *zs)
            for o in out:
                o.block_until_ready()
            dt = _t.perf_counter() - t0
            best = dt if best is None else min(best, dt)
        return best

    def fetch(self, out):
        res = []
        for c in range(self.n_cores):
            d = {}
            for i, n in enumerate(self.out_names):
                full = np.asarray(out[i])
                d[n] = full.reshape(self.n_cores, full.shape[0] // self.n_cores,
                                    *full.shape[1:])[c]
            res.append(d)
        return res


def _floor_runner():
    """Tiny copy program to calibrate the axon dispatch floor."""
    if "floor" in _CACHE:
        return _CACHE["floor"]
    from concourse import bacc, mybir
    import concourse.tile as tile
    f32 = mybir.dt.float32
    nc = bacc.Bacc("TRN2", target_bir_lowering=False, debug=False, num_devices=CORES)
    a = nc.dram_tensor("a", [P, P], f32, kind="ExternalInput")
    b = nc.dram_tensor("b", [P, P], f32, kind="ExternalOutput")
    with tile.TileContext(nc) as tc:
        with tc.tile_pool(name="p", bufs=1) as pool:
            t = pool.tile([P, P], f32)
            nc.sync.dma_start(out=t[:], in_=a[:])
            nc.sync.dma_start(out=b[:], in_=t[:])
    nc.compile()
    r = _PjrtRunner(nc, CORES)
    r.put_inputs([dict(a=np.zeros((P, P), np.float32)) for _ in range(CORES)])
    r.execute()
    _CACHE["floor"] = r
    return r


def run(inputs, cfg=None):
    global LAST_EXEC_NS, LAST_FLOOR_NS
    # bf16 tables measured speed-identical to f32 (gather is descriptor-bound,
    # not byte-bound) — default to f32 for bit-level accuracy at no cost.
    os.environ.setdefault("KERNEL_BF16", "0")
    if cfg is None:
        cfg = Cfg(N)
    feature = np.asarray(inputs["feature"], np.float32)
    k = int(np.asarray(inputs["low_gamma"]).shape[0])

    plan = plan_graph(cfg, inputs["src"], inputs["dst"])
    lens = dict(idx1=len(plan["idx1"][0]), ldw1=len(plan["ld1"][0]),
                idx2=len(plan["idx2"][0]), ldw2=len(plan["ld2"][0]))

    key = (cfg.N, cfg.D, lens["idx1"], lens["ldw1"], lens["idx2"], lens["ldw2"],
           os.environ.get("KERNEL_BF16", "0"), os.environ.get("KERNEL_GCOLS", "2"),
           os.environ.get("KERNEL_PROBE", ""),
           tuple(m["idx_off"] for m in plan["meta1"]),
           tuple(m["idx_off"] for m in plan["meta2"]))
    if key not in _CACHE:
        nc = build_program(cfg, plan["meta1"], plan["meta2"], lens)
        _CACHE[key] = _PjrtRunner(nc, cfg.CORES)
    runner = _CACHE[key]

    in_maps = _build_in_maps(cfg, plan, feature, inputs["snorm_n"],
                             inputs["W_low"], inputs["W_mid"], inputs["W_high"],
                             inputs["low_gamma"], inputs["mid_gamma"],
                             inputs["high_gamma"], inputs["bias"], k)
    global LAST_RUNNER
    LAST_RUNNER = runner
    runner.put_inputs(in_maps)
    out_arrs = runner.execute()
    if TIME_REPS > 0:
        t_k = runner.timed(TIME_REPS)
        fl = _floor_runner()
        t_f = fl.timed(TIME_REPS)
        LAST_EXEC_NS = int((t_k - t_f) * 1e9)
        LAST_FLOOR_NS = int(t_f * 1e9)
    res = runner.fetch(out_arrs)
    out = np.concatenate([res[c]["out"] for c in range(cfg.CORES)], axis=0)
    return out


def kernel(**inputs):
    return run(inputs)
